# revision 11
# baseline (speedup 1.0000x reference)
"""Trainium2 Bass kernel for nn_DecoderLayer_39221641347757.

Strategy (8 NeuronCores, SPMD single NEFF):
  - Attention: tensor-parallel over the 8 heads (head i -> core i).
    Per core: LN(x) -> transpose -> q/k/v projections (bf16 matmuls),
    exp-scores computed in BOTH [q,p] and [p,q] layouts (second matmul
    instead of transposing attn), causal masking via precomputed 0/1
    mask tiles, softmax denominator folded into the W_O output.
  - x_att partials reduce-scattered on-device (each core keeps its
    1/8 token shard), residual added from a host-sliced x shard.
  - MLP: token-parallel (512 tokens per core), full w1/w2 in bf16.
  - Outputs: per-head attn [B,T,T] per core (host stacks to [B,H,T,T]),
    per-shard x [B,T/8,D] per core (host concatenates).
"""

import sys

if "/opt/trn_rl_repo" not in sys.path:
    sys.path.insert(0, "/opt/trn_rl_repo")

from contextlib import ExitStack

import ml_dtypes
import numpy as np

import concourse.bass as bass
import concourse.mybir as mybir
import concourse.tile as tile
from concourse import bacc
from concourse.masks import make_identity

F32 = mybir.dt.float32
BF16 = mybir.dt.bfloat16
P = 128
NCORES = 8
LN_EPS = 1e-5

# Full-size problem dims
FULL = dict(B=2, T=2048, D=1024, DH=1024, F=4096)


def build_program(B, T, D, DH, F, ncores=NCORES, use_cc=True):
    """Build the SPMD Bass program (same NEFF for all cores)."""
    assert T % 512 == 0 and D % 512 == 0 and DH % 512 == 0 and F % 512 == 0
    TS_B = T // ncores            # per-batch token shard
    assert TS_B % P == 0
    TOK_SH = B * TS_B             # total shard tokens per core
    DC = D // P                   # d chunks
    HC = DH // P                  # head-dim chunks
    FC = F // P                   # ffn chunks
    QI = T // P                   # 128-row query chunks per batch
    NPJ = T // 512                # 512-wide key tiles per batch
    NDN = D // 512
    NHN = DH // 512
    SM_SCALE = 1.0 / float(np.sqrt(DH))

    nc = bacc.Bacc(None, target_bir_lowering=False, num_devices=ncores)

    # ---- I/O ----
    x_all = nc.dram_tensor("x_all", [B * T, D], F32, kind="ExternalInput")
    x_shard = nc.dram_tensor("x_shard", [B, TS_B, D], F32, kind="ExternalInput")
    wq_t = nc.dram_tensor("wq_t", [D, DH], BF16, kind="ExternalInput")
    wk_t = nc.dram_tensor("wk_t", [D, DH], BF16, kind="ExternalInput")
    wv_t = nc.dram_tensor("wv_t", [D, DH], BF16, kind="ExternalInput")
    wo_t = nc.dram_tensor("wo_t", [DH, D], BF16, kind="ExternalInput")
    w1_t = nc.dram_tensor("w1_t", [D, F], BF16, kind="ExternalInput")
    w2_t = nc.dram_tensor("w2_t", [F, D], BF16, kind="ExternalInput")
    b1_col = nc.dram_tensor("b1_col", [P, FC], F32, kind="ExternalInput")
    b2_v = nc.dram_tensor("b2_v", [D], F32, kind="ExternalInput")
    ln_scale = nc.dram_tensor("ln_scale", [D], F32, kind="ExternalInput")
    ln_bias = nc.dram_tensor("ln_bias", [D], F32, kind="ExternalInput")

    attn_out = nc.dram_tensor("attn_out", [B, T, T], F32, kind="ExternalOutput")
    x_out = nc.dram_tensor("x_out", [B, TS_B, D], F32, kind="ExternalOutput")

    AL = mybir.AluOpType
    AF = mybir.ActivationFunctionType
    AX = mybir.AxisListType

    with tile.TileContext(nc) as tc, ExitStack() as top:
        consts = top.enter_context(tc.tile_pool(name="consts", bufs=1))
        dram = top.enter_context(tc.tile_pool(name="dram", bufs=1, space="DRAM"))

        # ---- constants ----
        def part_bcast(ap):
            return bass.AP(tensor=ap.tensor, offset=ap.offset,
                           ap=[[0, P]] + list(ap.ap))

        ident = consts.tile([P, P], BF16)
        make_identity(nc, ident[:])
        scale_bc = consts.tile([P, D], F32)
        nc.gpsimd.dma_start(out=scale_bc[:], in_=part_bcast(ln_scale[:]))
        bias_bc = consts.tile([P, D], F32)
        nc.gpsimd.dma_start(out=bias_bc[:], in_=part_bcast(ln_bias[:]))
        b2_bc = consts.tile([P, D], F32)
        nc.gpsimd.dma_start(out=b2_bc[:], in_=part_bcast(b2_v[:]))
        b1c = consts.tile([P, FC], F32)
        nc.gpsimd.dma_start(out=b1c[:], in_=b1_col[:])
        eps_t = consts.tile([P, 1], F32)
        nc.vector.memset(eps_t[:], LN_EPS)
        zero_t = consts.tile([P, 512], F32)
        nc.vector.memset(zero_t[:], 0.0)
        # causal 0/1 masks for the 4 possible diagonal offsets (q-major and
        # p-major layouts).  maskq[d][r,c] = 1 iff c <= r + 128*d  (keep p<=q)
        # maskt[d][r,c] = 1 iff c >= r + 128*d
        maskq = consts.tile([P, 4, 512], F32)
        maskt = consts.tile([P, 4, 512], BF16)
        for d4 in range(4):
            nc.vector.memset(maskq[:, d4, :], 1.0)
            nc.gpsimd.affine_select(
                out=maskq[:, d4, :], in_=maskq[:, d4, :],
                compare_op=AL.is_ge, fill=0.0,
                base=128 * d4, pattern=[[-1, 512]], channel_multiplier=1,
            )
            nc.vector.memset(maskt[:, d4, :], 1.0)
            nc.gpsimd.affine_select(
                out=maskt[:, d4, :], in_=maskt[:, d4, :],
                compare_op=AL.is_ge, fill=0.0,
                base=-128 * d4, pattern=[[1, 512]], channel_multiplier=-1,
            )

        # ---- DRAM bounce buffers for the collective ----
        xatt_d = []
        rs_d = []
        for b in range(B):
            xa = dram.tile([T, D], F32, name=f"xatt{b}")
            xatt_d.append(xa)
            ro = dram.tile([TS_B, D], F32, name=f"rsout{b}")
            rs_d.append(ro)

        def layer_norm(pool, xt, tag):
            """LN of a [128, D] f32 tile -> new bf16 tile (y)."""
            nsub = D // 512
            stats = pool.tile([P, max(nsub, 1), 6], F32, tag=f"st{tag}")
            for s in range(nsub):
                nc.vector.bn_stats(out=stats[:, s, :], in_=xt[:, s * 512:(s + 1) * 512])
            mv = pool.tile([P, 2], F32, tag=f"mv{tag}")
            nc.vector.bn_aggr(out=mv[:], in_=stats[:])
            # mv[:,0] = mean, mv[:,1] = var -> rstd
            nc.scalar.activation(out=mv[:, 1:2], in_=mv[:, 1:2], func=AF.Sqrt,
                                 bias=eps_t[:], scale=1.0)
            nc.vector.reciprocal(out=mv[:, 1:2], in_=mv[:, 1:2])
            yf = pool.tile([P, D], F32, tag=f"yf{tag}")
            nc.vector.tensor_scalar(
                out=yf[:], in0=xt[:], scalar1=mv[:, 0:1], scalar2=mv[:, 1:2],
                op0=AL.subtract, op1=AL.mult,
            )
            nc.vector.tensor_mul(out=yf[:], in0=yf[:], in1=scale_bc[:])
            yb = pool.tile([P, D], BF16, tag=f"yb{tag}")
            nc.gpsimd.tensor_add(out=yb[:], in0=yf[:], in1=bias_bc[:])
            return yb

        # ================= attention (per batch) =================
        for b in range(B):
            with ExitStack() as bs:
                qkv = bs.enter_context(tc.tile_pool(name=f"qkv{b}", bufs=1))
                qT = qkv.tile([P, HC, T], BF16)
                kT = qkv.tile([P, HC, T], BF16)
                vv = qkv.tile([P, T // P, DH], BF16)

                # ---- phase A: LN + transpose + projections ----
                with ExitStack() as ph:
                    pA = ph.enter_context(tc.tile_pool(name=f"phA{b}", bufs=2))
                    pyT = ph.enter_context(tc.tile_pool(name=f"yT{b}", bufs=1))
                    pw = ph.enter_context(tc.tile_pool(name=f"wA{b}", bufs=2))
                    psA = ph.enter_context(
                        tc.tile_pool(name=f"psA{b}", bufs=3, space="PSUM"))
                    psT = ph.enter_context(
                        tc.tile_pool(name=f"psT{b}", bufs=2, space="PSUM"))

                    yT = pyT.tile([P, DC, T], BF16)
                    for ti in range(T // P):
                        xt = pA.tile([P, D], F32, tag="xt")
                        nc.sync.dma_start(
                            out=xt[:], in_=x_all[b * T + ti * P: b * T + (ti + 1) * P, :])
                        yb = layer_norm(pA, xt, "a")
                        for dc in range(DC):
                            pt = psT.tile([P, P], BF16, tag="pt")
                            nc.tensor.transpose(
                                pt[:], yb[:, dc * P:(dc + 1) * P], ident[:])
                            nc.vector.tensor_copy(
                                out=yT[:, dc, ti * P:(ti + 1) * P], in_=pt[:])

                    # q/k projections -> [h, tok] layout
                    for name, wdram, dest in (("q", wq_t, qT), ("k", wk_t, kT)):
                        wsb = pw.tile([P, DC, DH], BF16, tag="w", name=f"w{name}sb")
                        nc.sync.dma_start(
                            out=wsb[:], in_=wdram[:].rearrange("(c p) h -> p c h", p=P))
                        for hc in range(HC):
                            for tt in range(T // 512):
                                ps = psA.tile([P, 512], F32, tag="ps")
                                for dc in range(DC):
                                    nc.tensor.matmul(
                                        ps[:],
                                        wsb[:, dc, hc * P:(hc + 1) * P],
                                        yT[:, dc, tt * 512:(tt + 1) * 512],
                                        start=(dc == 0), stop=(dc == DC - 1))
                                nc.vector.tensor_copy(
                                    out=dest[:, hc, tt * 512:(tt + 1) * 512], in_=ps[:])
                    # v projection -> [tok, h] layout
                    wvsb = pw.tile([P, DC, DH], BF16, tag="w")
                    nc.sync.dma_start(
                        out=wvsb[:], in_=wv_t[:].rearrange("(c p) h -> p c h", p=P))
                    for pc in range(T // P):
                        for hn in range(NHN):
                            ps = psA.tile([P, 512], F32, tag="ps")
                            for dc in range(DC):
                                nc.tensor.matmul(
                                    ps[:],
                                    yT[:, dc, pc * P:(pc + 1) * P],
                                    wvsb[:, dc, hn * 512:(hn + 1) * 512],
                                    start=(dc == 0), stop=(dc == DC - 1))
                            nc.vector.tensor_copy(
                                out=vv[:, pc, hn * 512:(hn + 1) * 512], in_=ps[:])

                # ---- phase B: scores, softmax, z, x_att ----
                with ExitStack() as ph:
                    pB = ph.enter_context(tc.tile_pool(name=f"phB{b}", bufs=2))
                    pwo = ph.enter_context(tc.tile_pool(name=f"wo{b}", bufs=1))
                    psS = ph.enter_context(
                        tc.tile_pool(name=f"psS{b}", bufs=2, space="PSUM"))
                    psS2 = ph.enter_context(
                        tc.tile_pool(name=f"psS2{b}", bufs=2, space="PSUM"))
                    psZ = ph.enter_context(
                        tc.tile_pool(name=f"psZ{b}", bufs=2, space="PSUM"))
                    psX = ph.enter_context(
                        tc.tile_pool(name=f"psX{b}", bufs=2, space="PSUM"))

                    wosb = pwo.tile([P, HC, D], BF16)
                    nc.sync.dma_start(
                        out=wosb[:], in_=wo_t[:].rearrange("(c p) d -> p c d", p=P))
                    recipR = pB.tile([P, QI], F32, tag="recip", bufs=1)

                    # -- expS ([q,p]) -> attn rows + softmax denominators --
                    for qi in range(QI):
                        npj = qi // 4 + 1
                        dpart = pB.tile([P, 4], F32, tag="dp", bufs=3)
                        es_list = []
                        for pj in range(npj):
                            ps = psS.tile([P, 512], F32, tag="ss")
                            for hc in range(HC):
                                nc.tensor.matmul(
                                    ps[:],
                                    qT[:, hc, qi * P:(qi + 1) * P],
                                    kT[:, hc, pj * 512:(pj + 1) * 512],
                                    start=(hc == 0), stop=(hc == HC - 1))
                            es = pB.tile([P, 512], F32, tag="es", bufs=7)
                            if pj == qi // 4:  # diagonal tile: mask + sum
                                nc.scalar.activation(
                                    out=es[:], in_=ps[:], func=AF.Exp, scale=SM_SCALE)
                                nc.vector.tensor_mul(
                                    out=es[:], in0=es[:], in1=maskq[:, qi % 4, :])
                                nc.vector.reduce_sum(
                                    out=dpart[:, pj:pj + 1], in_=es[:], axis=AX.X)
                            else:
                                nc.scalar.activation(
                                    out=es[:], in_=ps[:], func=AF.Exp, scale=SM_SCALE,
                                    accum_out=dpart[:, pj:pj + 1])
                            es_list.append(es)
                        dsum = pB.tile([P, 1], F32, tag="dsum", bufs=3)
                        nc.vector.reduce_sum(out=dsum[:], in_=dpart[:, :npj], axis=AX.X)
                        nc.vector.reciprocal(out=recipR[:, qi:qi + 1], in_=dsum[:])
                        for pj, es in enumerate(es_list):
                            nc.vector.tensor_scalar_mul(
                                out=es[:], in0=es[:], scalar1=recipR[:, qi:qi + 1])
                            nc.sync.dma_start(
                                out=attn_out[b, qi * P:(qi + 1) * P, pj * 512:(pj + 1) * 512],
                                in_=es[:])
                        for zj in range(npj, NPJ):
                            nc.sync.dma_start(
                                out=attn_out[b, qi * P:(qi + 1) * P, zj * 512:(zj + 1) * 512],
                                in_=zero_t[:])

                    # -- expST ([p,q]) -> z -> x_att --
                    for Qj in range(T // 512):
                        npc = 4 * (Qj + 1)
                        est_list = []
                        for pc in range(npc):
                            ps = psS2.tile([P, 512], F32, tag="s2")
                            for hc in range(HC):
                                nc.tensor.matmul(
                                    ps[:],
                                    kT[:, hc, pc * P:(pc + 1) * P],
                                    qT[:, hc, Qj * 512:(Qj + 1) * 512],
                                    start=(hc == 0), stop=(hc == HC - 1))
                            est = pB.tile([P, 512], BF16, tag="est", bufs=18)
                            nc.scalar.activation(
                                out=est[:], in_=ps[:], func=AF.Exp, scale=SM_SCALE)
                            if pc // 4 == Qj:  # diagonal: zero q < p
                                nc.vector.tensor_mul(
                                    out=est[:], in0=est[:], in1=maskt[:, pc % 4, :])
                            est_list.append(est)
                        zT = pB.tile([P, HC, 512], BF16, tag="zt", bufs=2)
                        for hc in range(HC):
                            psz = psZ.tile([P, 512], F32, tag="zp")
                            for pc in range(npc):
                                nc.tensor.matmul(
                                    psz[:],
                                    vv[:, pc, hc * P:(hc + 1) * P],
                                    est_list[pc][:],
                                    start=(pc == 0), stop=(pc == npc - 1))
                            nc.vector.tensor_copy(out=zT[:, hc, :], in_=psz[:])
                        for qq in range(4):
                            qi = 4 * Qj + qq
                            for dn in range(NDN):
                                px = psX.tile([P, 512], F32, tag="xp")
                                for hc in range(HC):
                                    nc.tensor.matmul(
                                        px[:],
                                        zT[:, hc, qq * P:(qq + 1) * P],
                                        wosb[:, hc, dn * 512:(dn + 1) * 512],
                                        start=(hc == 0), stop=(hc == HC - 1))
                                xo = pB.tile([P, 512], F32, tag="xo", bufs=4)
                                nc.vector.tensor_scalar_mul(
                                    out=xo[:], in0=px[:], scalar1=recipR[:, qi:qi + 1])
                                nc.sync.dma_start(
                                    out=xatt_d[b][qi * P:(qi + 1) * P, dn * 512:(dn + 1) * 512],
                                    in_=xo[:])

            # reduce-scatter this batch's x_att partials across the 8 cores
            if use_cc:
                nc.gpsimd.collective_compute(
                    "ReduceScatter",
                    AL.add,
                    replica_groups=[list(range(ncores))],
                    ins=[xatt_d[b][:]],
                    outs=[rs_d[b][:]],
                )
            else:
                # debug stub: no cross-core reduction (numerically wrong)
                nc.gpsimd.dma_start(out=rs_d[b][:], in_=xatt_d[b][0:TS_B, :])

        # ================= MLP on the token shard =================
        with ExitStack() as ms:
            pM = ms.enter_context(tc.tile_pool(name="mlp", bufs=1))
            pMs = ms.enter_context(tc.tile_pool(name="mlps", bufs=2))
            psMT = ms.enter_context(tc.tile_pool(name="psMT", bufs=2, space="PSUM"))
            psH = ms.enter_context(tc.tile_pool(name="psH", bufs=2, space="PSUM"))
            psO = ms.enter_context(tc.tile_pool(name="psO", bufs=2, space="PSUM"))

            y2T = pM.tile([P, DC, TOK_SH], BF16)
            xn_all = pM.tile([P, TOK_SH // P, D], F32)
            NCB = TS_B // P  # 128-chunks per batch shard
            for b in range(B):
                rs_sb = pMs.tile([P, NCB, D], F32, tag="rs")
                nc.sync.dma_start(
                    out=rs_sb[:], in_=rs_d[b][:].rearrange("(c p) d -> p c d", p=P))
                xs_sb = pMs.tile([P, NCB, D], F32, tag="xs")
                nc.sync.dma_start(
                    out=xs_sb[:], in_=x_shard[b].rearrange("(c p) d -> p c d", p=P))
                for c in range(NCB):
                    tci = b * NCB + c
                    nc.vector.tensor_add(
                        out=xn_all[:, tci, :], in0=rs_sb[:, c, :], in1=xs_sb[:, c, :])
                    y2b = layer_norm(pMs, xn_all[:, tci, :], "m")
                    for dc in range(DC):
                        pt = psMT.tile([P, P], BF16, tag="pt2")
                        nc.tensor.transpose(
                            pt[:], y2b[:, dc * P:(dc + 1) * P], ident[:])
                        nc.vector.tensor_copy(
                            out=y2T[:, dc, tci * P:(tci + 1) * P], in_=pt[:])

            hT = pM.tile([P, FC, TOK_SH], BF16)
            for fs in range(F // 512):
                w1sb = pMs.tile([P, DC, 512], BF16, tag="w1")
                nc.sync.dma_start(
                    out=w1sb[:],
                    in_=w1_t[:].rearrange("(c p) f -> p c f", p=P)[:, :, fs * 512:(fs + 1) * 512])
                for ff in range(4):
                    fc = fs * 4 + ff
                    phh = psH.tile([P, TOK_SH], F32, tag="ph")
                    for dc in range(DC):
                        nc.tensor.matmul(
                            phh[:],
                            w1sb[:, dc, ff * P:(ff + 1) * P],
                            y2T[:, dc, :],
                            start=(dc == 0), stop=(dc == DC - 1))
                    nc.scalar.activation(
                        out=hT[:, fc, :], in_=phh[:], func=AF.Relu,
                        bias=b1c[:, fc:fc + 1], scale=1.0)

            NTC = TOK_SH // P
            for dn in range(NDN):
                po_tiles = [psO.tile([P, 512], F32, tag=f"po{t}", bufs=1,
                                     name=f"po_{dn}_{t}")
                            for t in range(NTC)]
                for fc in range(FC):
                    w2sl = pMs.tile([P, 512], BF16, tag="w2sl", bufs=3)
                    nc.sync.dma_start(
                        out=w2sl[:],
                        in_=w2_t[:].rearrange("(c p) d -> p c d", p=P)[:, fc, dn * 512:(dn + 1) * 512])
                    for tci in range(NTC):
                        nc.tensor.matmul(
                            po_tiles[tci][:],
                            hT[:, fc, tci * P:(tci + 1) * P],
                            w2sl[:],
                            start=(fc == 0), stop=(fc == FC - 1))
                for tci in range(NTC):
                    b = tci // NCB
                    rb = tci % NCB
                    ot = pMs.tile([P, 512], F32, tag="ot", bufs=3)
                    nc.vector.tensor_add(
                        out=ot[:], in0=po_tiles[tci][:],
                        in1=xn_all[:, tci, dn * 512:(dn + 1) * 512])
                    nc.vector.tensor_add(
                        out=ot[:], in0=ot[:], in1=b2_bc[:, dn * 512:(dn + 1) * 512])
                    nc.sync.dma_start(
                        out=x_out[b, rb * P:(rb + 1) * P, dn * 512:(dn + 1) * 512],
                        in_=ot[:])

    nc.finalize()
    return nc


def make_in_maps(inputs, B, T, D, DH, F, ncores=NCORES):
    """Host-side prep: slice/transposes/casts per core."""
    bf = ml_dtypes.bfloat16
    x = np.ascontiguousarray(np.asarray(inputs["x"], np.float32))
    W_K = np.asarray(inputs["W_K"], np.float32)
    W_Q = np.asarray(inputs["W_Q"], np.float32)
    W_V = np.asarray(inputs["W_V"], np.float32)
    W_O = np.asarray(inputs["W_O"], np.float32)
    w1 = np.asarray(inputs["w1"], np.float32)
    w2 = np.asarray(inputs["w2"], np.float32)
    b1 = np.asarray(inputs["b1"], np.float32)
    b2 = np.asarray(inputs["b2"], np.float32)
    lsc = np.asarray(inputs["ln_scale"], np.float32)
    lbi = np.asarray(inputs["ln_bias"], np.float32)

    TS_B = T // ncores
    FC = F // P
    x_all = np.ascontiguousarray(x.reshape(B * T, D))
    w1t = np.ascontiguousarray(w1.T.astype(bf))
    w2t = np.ascontiguousarray(w2.T.astype(bf))
    b1c = np.ascontiguousarray(b1.reshape(FC, P).T.astype(np.float32))

    in_maps = []
    for i in range(ncores):
        m = {
            "x_all": x_all,
            "x_shard": np.ascontiguousarray(
                x[:, i * TS_B:(i + 1) * TS_B, :]),
            "wq_t": np.ascontiguousarray(W_Q[i].T.astype(bf)),
            "wk_t": np.ascontiguousarray(W_K[i].T.astype(bf)),
            "wv_t": np.ascontiguousarray(W_V[i].T.astype(bf)),
            "wo_t": np.ascontiguousarray(W_O[:, :, i].T.astype(bf)),
            "w1_t": w1t,
            "w2_t": w2t,
            "b1_col": b1c,
            "b2_v": b2,
            "ln_scale": lsc,
            "ln_bias": lbi,
        }
        in_maps.append(m)
    return in_maps


_PROG_CACHE = {}


def _get_program(dims):
    key = tuple(sorted(dims.items()))
    if key not in _PROG_CACHE:
        _PROG_CACHE[key] = build_program(**dims)
    return _PROG_CACHE[key]


def assemble_outputs(results, B, T, D, ncores=NCORES):
    TS_B = T // ncores
    attn = np.empty((B, ncores, T, T), np.float32)
    xf = np.empty((B, T, D), np.float32)
    for i, r in enumerate(results):
        attn[:, i] = r["attn_out"]
        xf[:, i * TS_B:(i + 1) * TS_B, :] = r["x_out"]
    return xf, attn


def kernel(**inputs):
    from concourse.bass_utils import run_bass_kernel_spmd

    dims = dict(FULL)
    nc = _get_program(dims)
    in_maps = make_in_maps(inputs, ncores=NCORES, **dims)
    res = run_bass_kernel_spmd(nc, in_maps, core_ids=list(range(NCORES)))
    return assemble_outputs(res.results, dims["B"], dims["T"], dims["D"])
